# revision 61
# baseline (speedup 1.0000x reference)
"""Trainium2 Bass kernel for block-causal sparse attention (MLA-style KV).

Sharding: tensor-parallel over heads. 16 heads / 8 cores = 2 heads per core,
one KV head per core-pair. Each core computes q/k/v projections from the full
(transposed) x, RoPE, sparse attention for its 2 heads, and a partial output
projection; the host sums the 8 partial outputs.

Sparsity structure (T=4096, BLOCK=128, WINDOW=512, GLOBAL_EVERY=64):
for query block b, visible keys are blocks b-4..b (block b-4 masked by a fixed
triangular+global pattern) plus "global" columns j%64==0 with j < 128*(b-4).

All matmul inputs are bfloat16 (fp32 PSUM accumulation). Scores are computed
transposed ([k, q] layout) so probabilities feed the PV and output-projection
matmuls with no transposes.

RoPE: q/k head-dim channels are permuted host-side so rotate-half pairs
(d, d+64) sit on adjacent partitions (2d, 2d+1); the swap is then a single
32-periodic DVE stream_shuffle instead of DMA round-trips. cos/sin tables are
permuted + sign-folded to match. Scores are invariant to the shared q/k
permutation.

Softmax denominators: each p tile is the STATIONARY matmul operand with a ones
column moving (cost ~= 1 PE row per 128-query chunk, vs the full tile width
for a ones-stationary reduction). Both heads accumulate into one PSUM bank;
the [q, 8] result is transposed back and partition-broadcast per chunk.

Scheduling: the two heads' attention item streams are interleaved on the PE
and the previous tile's output projection is emitted between items as filler,
so the PE never waits on the exp->mask->PV producer chain. DMAs are batched
into few multi-dimensional transfers (HWDGE charges ~625ns per DMA
instruction regardless of size).
"""

import numpy as np

N_CORES = 8
T = 4096
C = 2048
L = 512
H = 16
KVH = 4
HD = 128
BLOCK = 128
WINDOW = 512
GLOBAL_EVERY = 64
ROPE_THETA = 10000.0

QTW = 512            # query tile width (4 blocks)
NQT = T // QTW       # 8
NKT = C // 128       # 16 contraction tiles for projections
NG = T // GLOBAL_EVERY  # 64 global columns

_CACHE = {}

SWAP_MASK = [i ^ 1 for i in range(32)]  # adjacent-pair partition swap


def _build_module():
    import concourse.bacc as bacc
    import concourse.mybir as mybir
    import concourse.tile as tile
    from contextlib import ExitStack

    F32 = mybir.dt.float32
    F32R = mybir.dt.float32r
    BF16 = mybir.dt.bfloat16
    F8 = mybir.dt.float8e4
    DR = mybir.MatmulPerfMode.DoubleRowSwInterleave
    EXP = mybir.ActivationFunctionType.Exp

    nc = bacc.Bacc("TRN2", target_bir_lowering=False, debug=False,
                   num_devices=N_CORES)

    # x and the q/kv projection weights ship as scaled fp8 hi/lo pairs for
    # DoubleRow matmuls (2 contraction tiles per call at 0.5 cycles/row).
    # Weight dram layout is the SBUF image: [128, pair, proj, two, 128].
    xth = nc.dram_tensor("xth", [C, T], F8, kind="ExternalInput")
    xtl = nc.dram_tensor("xtl", [C, T], F8, kind="ExternalInput")
    wqh = nc.dram_tensor("wqh", [128, 4096], F8, kind="ExternalInput")
    wql = nc.dram_tensor("wql", [128, 4096], F8, kind="ExternalInput")
    wkvh = nc.dram_tensor("wkvh", [128, 4096], F8, kind="ExternalInput")
    wkvl = nc.dram_tensor("wkvl", [128, 4096], F8, kind="ExternalInput")
    wo = nc.dram_tensor("wo", [2 * HD, C], BF16, kind="ExternalInput")
    csd = nc.dram_tensor("csd", [HD, 2 * T], BF16, kind="ExternalInput")
    maskt = nc.dram_tensor("maskt", [128, 128], BF16, kind="ExternalInput")
    maskg = nc.dram_tensor("maskg", [NG, T], BF16, kind="ExternalInput")
    onesd = nc.dram_tensor("onesd", [128, 1], BF16, kind="ExternalInput")
    identd = nc.dram_tensor("identd", [128, 128], BF16, kind="ExternalInput")
    identrd = nc.dram_tensor("identrd", [128, 128], F32R, kind="ExternalInput")
    out = nc.dram_tensor("out", [T, C], BF16, kind="ExternalOutput")

    scale = 1.0 / np.sqrt(HD)

    with tile.TileContext(nc) as tc, ExitStack() as ctx:
        res = ctx.enter_context(tc.tile_pool(name="res", bufs=1))
        kT = res.tile([128, T], BF16, tag="kT")
        vN = res.tile([128, T], BF16, tag="vN")
        kG = res.tile([128, NG], BF16, tag="kG")
        vG = res.tile([64, 128], BF16, tag="vG")
        vGT = res.tile([128, NG], BF16, tag="vGT")
        mT = res.tile([128, 128], BF16, tag="mT")
        mG = res.tile([NG, T], BF16, tag="mG")
        ones = res.tile([128, 1], BF16, tag="ones")
        ident = res.tile([128, 128], BF16, tag="ident")
        identr = res.tile([128, 128], F32R, tag="identr")
        cs = res.tile([128, 2 * T], BF16, tag="cs")
        wo_sb = res.tile([128, 2 * C], BF16, tag="wo_sb")
        wq_h = res.tile([128, 4096], F8, tag="wq_h")
        wq_l = res.tile([128, 4096], F8, tag="wq_l")
        wkv_h = res.tile([128, 4096], F8, tag="wkv_h")
        wkv_l = res.tile([128, 4096], F8, tag="wkv_l")

        xpool = ctx.enter_context(tc.tile_pool(name="xpool", bufs=4))
        qlp = ctx.enter_context(tc.tile_pool(name="qlp", bufs=3))
        vtp = ctx.enter_context(tc.tile_pool(name="vtp", bufs=3))
        swp = ctx.enter_context(tc.tile_pool(name="swp", bufs=3))
        tmpp = ctx.enter_context(tc.tile_pool(name="tmpp", bufs=4))
        ppool = ctx.enter_context(tc.tile_pool(name="ppool", bufs=8))
        ynp = ctx.enter_context(tc.tile_pool(name="ynp", bufs=3))
        recp = ctx.enter_context(tc.tile_pool(name="recp", bufs=2))
        rbcp = ctx.enter_context(tc.tile_pool(name="rbcp", bufs=6))
        obp = ctx.enter_context(tc.tile_pool(name="obp", bufs=4))
        dsbp = ctx.enter_context(tc.tile_pool(name="dsbp", bufs=3))

        # merged PSUM pool for projection passes, score tiles and transposes
        psp = ctx.enter_context(tc.tile_pool(name="psp", bufs=3, space="PSUM"))
        ypool = ctx.enter_context(tc.tile_pool(name="ypool", bufs=2, space="PSUM"))
        dpool = ctx.enter_context(tc.tile_pool(name="dpool", bufs=1, space="PSUM"))
        opool = ctx.enter_context(tc.tile_pool(name="opool", bufs=2, space="PSUM"))

        # ---- one-time loads (batched) ----
        nc.gpsimd.dma_start(ident[:], identd[:])
        nc.gpsimd.dma_start(identr[:], identrd[:])
        nc.gpsimd.dma_start(mT[:], maskt[:])
        nc.gpsimd.dma_start(ones[:], onesd[:])
        # tile-0 critical-path loads first, finely sliced so the first
        # projection matmuls start ~1us in; cos/sin next (first rope), bulk
        # weights after
        nc.sync.dma_start(wq_h[:, 0:512], wqh[:, 0:512])
        nc.sync.dma_start(wq_h[:, 512:], wqh[:, 512:])

        # pending output projection: {"ynorm", "qs0", "n" (next unit 0..16),
        # "ob" (current staging tile)}
        pending_wo = {"state": None}
        pending_norm = {"state": None}

        def flush_norm():
            """Finish the previous tile's normalize (transpose -> recip ->
            broadcast -> y/d) and queue its output projection. Emitted inside
            the next tile's projection phase so the PE-side transpose never
            stalls on the DVE's d copy."""
            st = pending_norm["state"]
            if st is None:
                return
            emit_wo_filler(16)  # finish any leftover wo units first
            pending_norm["state"] = None
            dsb, y_prev, qs0_prev = st
            # transpose each head's d columns onto partition 0 of a PSUM
            # row (partition_broadcast can only read partition-0 inputs),
            # then reciprocal to SBUF and broadcast per chunk
            rec = [None, None]
            for h in range(2):
                dtp = psp.tile([128, 512], F32R, tag="ps", name=f"dtp{h}")
                for cj in range(4):
                    nc.tensor.matmul(dtp[0:1, cj * 128:(cj + 1) * 128],
                                     dsb[:, 4 * h + cj:4 * h + cj + 1],
                                     identr[:], is_transpose=True,
                                     start=(cj == 0), stop=(cj == 3))
                rec[h] = recp.tile([1, 512], F32, tag=f"rec{h}",
                                   name=f"rec{h}")
                nc.vector.reciprocal(rec[h][:], dtp[0:1, :512])
            ynorm = [ynp.tile([128, QTW], BF16, tag=f"yn{h}", name=f"yn{h}")
                     for h in range(2)]
            for cj in range(4):
                for h in range(2):
                    rbc = rbcp.tile([128, 128], F32, tag="rbc")
                    nc.gpsimd.partition_broadcast(
                        rbc[:], rec[h][0:1, cj * 128:(cj + 1) * 128])
                    nc.vector.tensor_mul(
                        ynorm[h][:, cj * 128:(cj + 1) * 128],
                        y_prev[h][:, cj * 128:(cj + 1) * 128], rbc[:])
            pending_wo["state"] = {"ynorm": ynorm, "qs0": qs0_prev, "n": 0,
                                   "ob": None}

        def emit_wo_filler(k=2):
            """Emit up to k (qs, n) units of the previous tile's output
            projection; used as PE stall filler inside the attention loop."""
            for _ in range(k):
                st = pending_wo["state"]
                if st is None or st["n"] >= 16:
                    return
                qs, nn = divmod(st["n"], 4)
                if nn == 0:
                    st["ob"] = obp.tile([128, 2048], BF16, tag="ob", name="ob")
                ynorm, qs0, ob = st["ynorm"], st["qs0"], st["ob"]
                rows = slice(qs0 + qs * 128, qs0 + (qs + 1) * 128)
                if st.get("mixpool") and st["n"] % 2:
                    # last tile: psp is idle, interleave its banks so the
                    # unit pipeline isn't paced by the psum->sbuf copies
                    o_ps = psp.tile([128, 512], F32, tag="ps", name="o_ps")
                else:
                    o_ps = opool.tile([128, 512], F32, tag="o", name="o_ps")
                nc.tensor.matmul(o_ps[:], ynorm[0][:, qs * 128:(qs + 1) * 128],
                                 wo_sb[:, nn * 512:nn * 512 + 512],
                                 start=True, stop=False)
                nc.tensor.matmul(o_ps[:], ynorm[1][:, qs * 128:(qs + 1) * 128],
                                 wo_sb[:, C + nn * 512:C + nn * 512 + 512],
                                 start=False, stop=True)
                if nn % 2 == 0:
                    nc.scalar.copy(ob[:, nn * 512:(nn + 1) * 512], o_ps[:])
                else:
                    nc.vector.tensor_copy(ob[:, nn * 512:(nn + 1) * 512],
                                          o_ps[:])
                if nn == 3:
                    nc.sync.dma_start(out[rows, :], ob[:])
                st["n"] += 1

        for it in range(NQT):
            nt = it
            b0 = 4 * it
            ts = slice(nt * 512, (nt + 1) * 512)
            qs0 = it * QTW

            # ---- batched input DMAs for this t-tile (fp8 hi/lo) ----
            xbh = xpool.tile([128, NKT * 512], F8, tag="xh", name="xbh")
            xbl = xpool.tile([128, NKT * 512], F8, tag="xl", name="xbl")
            xh_groups = ([(0, 2), (2, 4), (4, 8), (8, 12), (12, 16)]
                         if it == 0 else [(0, 8), (8, 16)])
            for g0, g1 in xh_groups:
                nc.sync.dma_start(
                    xbh[:, g0 * 512:g1 * 512].rearrange(
                        "p (a b) -> p a b", a=g1 - g0),
                    xth.rearrange("(a p) t -> p a t", p=128)[:, g0:g1, ts])
            if it == 0:
                # cos/sin tile-0 slices: first rope is ~8us in, well after x
                nc.sync.dma_start(cs[:, 0:512], csd[:, 0:512])
                nc.sync.dma_start(cs[:, T:T + 512], csd[:, T:T + 512])
                nc.sync.dma_start(wq_l[:], wql[:, :])
            for g in range(2):
                nc.sync.dma_start(
                    xbl[:, g * 4096:(g + 1) * 4096].rearrange(
                        "p (a b) -> p a b", a=8),
                    xtl.rearrange("(a p) t -> p a t", p=128)[
                        :, g * 8:(g + 1) * 8, ts])
                if it == 0:
                    nc.sync.dma_start([wkv_h, wkv_l][g][:],
                                      [wkvh, wkvl][g][:, :])
            if it == 0:
                nc.gpsimd.dma_start(mG[:], maskg[:])
                nc.sync.dma_start(cs[:, 512:1024], csd[:, 512:1024])
                nc.sync.dma_start(cs[:, T + 512:T + 1024],
                                  csd[:, T + 512:T + 1024])
            elif it == 1:
                # behind this tile's x in the DMA queue on purpose
                nc.sync.dma_start(
                    wo_sb[:].rearrange("p (a b) -> p a b", a=2),
                    wo.rearrange("(a p) b -> p a b", p=128))
                nc.sync.dma_start(cs[:, 1024:T], csd[:, 1024:T])
                nc.sync.dma_start(cs[:, T + 1024:], csd[:, T + 1024:])

            cos_t = cs[:, ts]
            sin_t = cs[:, T + nt * 512:T + (nt + 1) * 512]


            qloc = [qlp.tile([128, 512], BF16, tag=f"ql{h}", name=f"ql{h}")
                    for h in range(2)]
            wsrc = [(wq_h, wq_l, 0), (wq_h, wq_l, 1),
                    (wkv_h, wkv_l, 0), (wkv_h, wkv_l, 1)]
            vT_t = vtp.tile([128, 512], BF16, tag="vT")

            def proj_term(pj, xb, wt, sub, start, stop):
                for g in range(8):
                    woff = g * 512 + sub * 256
                    nc.tensor.matmul(
                        pj[:],
                        wt[:, woff:woff + 256].rearrange(
                            "p (two m) -> p two m", two=2),
                        xb[:, g * 1024:(g + 1) * 1024].rearrange(
                            "p (two n) -> p two n", two=2),
                        start=(start and g == 0), stop=(stop and g == 7),
                        perf_mode=DR)

            pjt = [None] * 4
            if it == 0:
                # DMA-feed-bound first tile: interleave the q0/q1 passes at
                # term granularity in data-arrival order (wqh+xh first, then
                # wql, then xl) so the PE always has something runnable
                pjt[0] = psp.tile([128, 512], F32, tag="ps", name="pj0")
                pjt[1] = psp.tile([128, 512], F32, tag="ps", name="pj1")
                for sub in range(2):
                    proj_term(pjt[sub], xbh, wq_h, sub, True, False)
                for sub in range(2):
                    proj_term(pjt[sub], xbh, wq_l, sub, False, False)
                for sub in range(2):
                    proj_term(pjt[sub], xbl, wq_h, sub, False, True)
            for i in range(4):
                wh_t, wl_t, sub = wsrc[i]
                if it == 0 and i < 2:
                    pj = pjt[i]
                else:
                    pj = psp.tile([128, 512], F32, tag="ps", name=f"pj{i}")
                    proj_term(pj, xbh, wh_t, sub, True, False)
                    proj_term(pj, xbh, wl_t, sub, False, False)
                    proj_term(pj, xbl, wh_t, sub, False, True)
                if i < 3:
                    # RoPE (pair-adjacent layout): dest = pj*cos + swap(pj)*sinS
                    dest = qloc[i][:] if i < 2 else kT[:, ts]
                    sw = swp.tile([128, 512], F32, tag="sw")
                    nc.vector.stream_shuffle(sw[:], pj[:], SWAP_MASK)
                    ta = tmpp.tile([128, 512], BF16, tag="ta")
                    nc.vector.tensor_mul(ta[:], pj[:], cos_t)
                    tb = tmpp.tile([128, 512], BF16, tag="tb")
                    nc.gpsimd.tensor_mul(tb[:], sw[:], sin_t)
                    nc.gpsimd.tensor_add(dest, ta[:], tb[:])
                else:
                    nc.scalar.copy(vT_t[:], pj[:])

            # finish the previous tile's normalize here: the PE transpose
            # comes after pass 3 (d copy long done), and the rope ops sit
            # ahead of the recip/muls in the DVE queue; ynorm is only needed
            # by the wo fillers ~10us later
            flush_norm()

            gsl = slice(nt * 8, (nt + 1) * 8)
            nc.vector.tensor_copy(kG[:, gsl], kT[:, ts][:, 0:512:GLOBAL_EVERY])
            nc.vector.tensor_copy(vGT[:, gsl], vT_t[:][:, 0:512:GLOBAL_EVERY])
            gw2 = 8 * (nt + 1)

            # ---- attention for query tile `it`, two heads interleaved ----
            gw = min(NG, 8 * it)   # written prefix of kG/vG; 0 for it=0
            # item order: the global item goes FIRST - it covers the full
            # query width (one PSUM start zeroes the whole bank) and only
            # touches previous tiles' kG/vG, so the QK stream never waits on
            # this tile's rope/transposes; then the window items (previous
            # tile's kT/vN), then diag + upper (this tile's).
            # items: (kb, qoff, w, tri) or "glob"
            if it == 0:
                items = [(b0, 0, 512, None)]
                for j in range(3):
                    items.append((j + 1, (j + 1) * 128, (3 - j) * 128, None))
            else:
                items = ["glob"]
                for j in range(4):
                    items.append((b0 - 4 + j, 0, (j + 1) * 128, j))
                items.append((b0, 0, 512, None))
                for j in range(3):
                    items.append((b0 + 1 + j, (j + 1) * 128, (3 - j) * 128, None))
            n_items = len(items)

            y_ps = [ypool.tile([128, QTW], F32, tag="y", name=f"y{h}")
                    for h in range(2)]
            d_ps = dpool.tile([128, 8], F32, tag="d")
            s_tiles = [[None] * n_items, [None] * n_items]

            def emit_qk(h, ii):
                s = psp.tile([128, QTW], F32, tag="ps", name="s")
                if items[ii] == "glob":
                    nc.tensor.matmul(s[:gw, :], kG[:, :gw], qloc[h][:],
                                     start=True, stop=True)
                else:
                    kb, qoff, w, _ = items[ii]
                    nc.tensor.matmul(
                        s[:, :w], kT[:, kb * 128:(kb + 1) * 128],
                        qloc[h][:, qoff:qoff + w],
                        start=True, stop=True)
                s_tiles[h][ii] = s

            def emit_rest(h, ii):
                first = ii == 0
                last = ii == n_items - 1
                s = s_tiles[h][ii]
                p = ppool.tile([128, QTW], BF16, tag="p")
                # d flags: one group for the whole [128, 8] bank across both
                # heads - start only on the very first d matmul (h0, ii0,
                # first column), stop only on the very last (h1, last item's
                # last column)
                if items[ii] == "glob":
                    nc.scalar.activation(p[:gw, :], s[:gw, :], EXP, scale=scale)
                    nc.vector.tensor_mul(p[:gw, :], p[:gw, :],
                                         mG[:gw, qs0:qs0 + QTW])
                    nc.tensor.matmul(y_ps[h][:, :], vG[:gw, :], p[:gw, :],
                                     start=first, stop=last)
                    for cj in range(4):
                        nc.tensor.matmul(
                            d_ps[:, 4 * h + cj:4 * h + cj + 1],
                            p[:gw, cj * 128:(cj + 1) * 128], ones[:gw, :],
                            start=(h == 0 and first and cj == 0),
                            stop=(h == 1 and last and cj == 3))
                else:
                    kb, qoff, w, tri = items[ii]
                    nc.scalar.activation(p[:, :w], s[:, :w], EXP, scale=scale)
                    if tri is not None:
                        nc.vector.tensor_mul(p[:, tri * 128:(tri + 1) * 128],
                                             p[:, tri * 128:(tri + 1) * 128],
                                             mT[:])
                    nc.tensor.matmul(y_ps[h][:, qoff:qoff + w],
                                     vN[:, kb * 128:(kb + 1) * 128], p[:, :w],
                                     start=first, stop=last)
                    cr = list(range(qoff // 128, (qoff + w) // 128))
                    for cj in cr:
                        c0 = cj * 128 - qoff
                        nc.tensor.matmul(
                            d_ps[:, 4 * h + cj:4 * h + cj + 1],
                            p[:, c0:c0 + 128], ones[:, :],
                            start=(h == 0 and first and cj == cr[0]),
                            stop=(h == 1 and last and cj == cr[-1]))

            emit_qk(0, 0)
            emit_qk(1, 0)
            emit_wo_filler(1)

            # v transpose for this t-tile: 4 transposes into one PSUM tile,
            # one bulk copy out (slotted here so the PE isn't waiting on the
            # DVE's vT psum->sbuf copy)
            tp = psp.tile([128, 512], BF16, tag="ps", name="tp")
            for j in range(4):
                nc.tensor.matmul(tp[:, j * 128:(j + 1) * 128],
                                 vT_t[:, j * 128:(j + 1) * 128], ident[:],
                                 is_transpose=True,
                                 start=(j == 0), stop=(j == 3))
            nc.scalar.copy(vN[:, nt * 512:(nt + 1) * 512], tp[:])

            emit_rest(0, 0)
            if n_items > 1:
                emit_qk(0, 1)
            emit_rest(1, 0)
            if n_items > 1:
                emit_qk(1, 1)
            emit_wo_filler(1)

            # incremental global V transpose (needed by the global item only)
            tpg = psp.tile([128, 512], BF16, tag="ps", name="tpg")
            nc.tensor.transpose(tpg[:gw2, :128], vGT[:, :gw2], ident[:])
            nc.vector.tensor_copy(vG[:gw2, :], tpg[:gw2, :128])

            # wo filler units weighted by the round's PE-work deficit:
            # rounds handling narrow (128/256-wide) items leave the PE
            # starving on the exp chain, wide rounds are self-covering
            for ii in range(1, n_items):
                emit_rest(0, ii)
                if ii + 1 < n_items:
                    emit_qk(0, ii + 1)
                else:
                    # no lookahead work left: extra units here cover the
                    # last item's exp->mask chain
                    emit_wo_filler(2)
                emit_rest(1, ii)
                if ii + 1 < n_items:
                    emit_qk(1, ii + 1)
                emit_wo_filler(2)

            # d -> sbuf first (ahead of the ob copies in the DVE queue) so the
            # next tile's normalize transpose isn't gated on it
            dsb = dsbp.tile([128, 8], F32R, tag="dsb")
            nc.vector.tensor_copy(dsb[:], d_ps[:])
            emit_wo_filler(3)
            if it < NQT - 1:
                pending_norm["state"] = (dsb, y_ps, qs0)
            else:
                # last tile: inline, emitting wo units per query chunk as it
                # is normalized, to shorten the end-of-kernel drain
                rec = [None, None]
                for h in range(2):
                    dtp = psp.tile([128, 512], F32R, tag="ps", name=f"dtp{h}")
                    for cj in range(4):
                        nc.tensor.matmul(
                            dtp[0:1, cj * 128:(cj + 1) * 128],
                            dsb[:, 4 * h + cj:4 * h + cj + 1], identr[:],
                            is_transpose=True,
                            start=(cj == 0), stop=(cj == 3))
                    rec[h] = recp.tile([1, 512], F32, tag=f"rec{h}",
                                       name=f"rec{h}")
                    nc.vector.reciprocal(rec[h][:], dtp[0:1, :512])
                ynorm = [ynp.tile([128, QTW], BF16, tag=f"yn{h}",
                                  name=f"yn{h}") for h in range(2)]
                pending_wo["state"] = {"ynorm": ynorm, "qs0": qs0, "n": 0,
                                       "ob": None, "mixpool": True}
                for cj in range(4):
                    for h in range(2):
                        rbc = rbcp.tile([128, 128], F32, tag="rbc")
                        nc.gpsimd.partition_broadcast(
                            rbc[:], rec[h][0:1, cj * 128:(cj + 1) * 128])
                        nc.vector.tensor_mul(
                            ynorm[h][:, cj * 128:(cj + 1) * 128],
                            y_ps[h][:, cj * 128:(cj + 1) * 128], rbc[:])
                    emit_wo_filler(4)

    nc.compile()
    return nc


def _host_inputs(x, w_q, w_kv_down, w_k_up, w_v_up, w_o):
    """Build the per-core input maps (host-side shard + precompute)."""
    import ml_dtypes
    BF = ml_dtypes.bfloat16
    E4 = ml_dtypes.float8_e4m3
    SX, SW = 16.0, 1024.0
    INV = 1.0 / (SX * SW)
    x = np.asarray(x)
    w_q = np.asarray(w_q)
    w_kv_down = np.asarray(w_kv_down)
    w_k_up = np.asarray(w_k_up)
    w_v_up = np.asarray(w_v_up)
    w_o = np.asarray(w_o)
    x2 = np.ascontiguousarray(x.reshape(T, C).astype(np.float32))
    xs = x2.T.astype(np.float32) * SX
    xh = xs.astype(E4)
    xl = (xs - xh.astype(np.float32)).astype(E4)
    xh = np.ascontiguousarray(xh)
    xl = np.ascontiguousarray(xl)

    def dr_layout(w):
        """[C, 256] f32 -> fp8 hi/lo pair in the DoubleRow SBUF image
        [128, pair, proj, two, 128] -> [128, 4096]."""
        ws = w * SW
        hi = ws.astype(E4)
        lo = (ws - hi.astype(np.float32)).astype(E4)
        outs = []
        for a in (hi, lo):
            # DoubleRowSwInterleave weight image: within each (pair, proj)
            # 256-column block, columns are [A127, B127, A126, B126, ...]
            # where A/B are the two k-tiles of the pair
            a = a.reshape(8, 2, 128, 2, 128)          # [g, two, p, i, m]
            a = a[:, :, :, :, ::-1]                   # reverse m
            a = a.transpose(2, 0, 3, 4, 1)            # [p, g, i, m, two]
            outs.append(np.ascontiguousarray(a.reshape(128, 4096)))
        return outs

    # RoPE channel permutation: pair (j, j+64) -> (2j, 2j+1) within each head
    perm = np.empty(HD, np.int64)
    perm[0::2] = np.arange(64)
    perm[1::2] = np.arange(64) + 64

    # RoPE tables in the permuted layout, sign folded into sin; cos|sin packed
    freqs = 1.0 / (ROPE_THETA ** (np.arange(0, HD, 2, dtype=np.float64) / HD))
    emb = np.arange(T, dtype=np.float64)[:, None] * freqs[None, :]   # [T, 64]
    cosP = np.empty((HD, T), np.float64)
    sinP = np.empty((HD, T), np.float64)
    cosP[0::2] = cosP[1::2] = np.cos(emb).T
    sinP[0::2] = -np.sin(emb).T
    sinP[1::2] = np.sin(emb).T
    # fold the fp8 descale (1/(SX*SW)) into the rope tables (q, k paths)
    csP = np.ascontiguousarray(
        (np.concatenate([cosP, sinP], axis=1) * INV).astype(BF))

    # fixed triangular+global mask for the b-4 key block, [k_off, q_off]
    oi = np.arange(128)
    mT = ((oi[None, :] <= oi[:, None]) | (oi[:, None] % 64 == 0)).astype(BF)

    # global-column mask [g, q]: visible iff 64 g < 128 (q//128 - 4)
    g = np.arange(NG)
    qb = np.arange(T) // BLOCK
    mG = (64 * g[:, None] < 128 * (qb[None, :] - 4)).astype(BF)

    onesv = np.ones((128, 1), BF)
    identv = np.eye(128, dtype=BF)
    identr32 = np.eye(128, dtype=np.float32)

    wk_f = (w_kv_down.astype(np.float32) @ w_k_up.astype(np.float32))  # [C, KVH*HD]
    wv_f = (w_kv_down.astype(np.float32) @ w_v_up.astype(np.float32))

    in_maps = []
    for c in range(N_CORES):
        h0 = 2 * c
        kv = h0 // (H // KVH)
        wq_c = w_q[:, h0 * HD:(h0 + 2) * HD].astype(np.float32).copy()
        # permute q channels within each head block, k channels within block
        wq_c = wq_c.reshape(C, 2, HD)[:, :, perm].reshape(C, 2 * HD)
        wk_c = wk_f[:, kv * HD:(kv + 1) * HD][:, perm]
        wv_c = wv_f[:, kv * HD:(kv + 1) * HD]
        wkv_c = np.concatenate(
            [wk_c.reshape(C, 1, HD), wv_c.reshape(C, 1, HD)],
            axis=1).reshape(C, 2 * HD)
        # fold the fp8 descale into w_o (v path carries SX*SW through PV)
        wo_c = w_o[h0 * HD:(h0 + 2) * HD, :].astype(np.float32) * INV
        wqh_c, wql_c = dr_layout(wq_c)
        wkvh_c, wkvl_c = dr_layout(wkv_c.astype(np.float32))
        in_maps.append({
            "xth": xh, "xtl": xl,
            "wqh": wqh_c, "wql": wql_c,
            "wkvh": wkvh_c, "wkvl": wkvl_c,
            "wo": np.ascontiguousarray(wo_c.astype(BF)),
            "csd": csP, "maskt": mT, "maskg": mG,
            "onesd": onesv, "identd": identv, "identrd": identr32,
        })
    return in_maps


def _get_module():
    if "nc" not in _CACHE:
        _CACHE["nc"] = _build_module()
    return _CACHE["nc"]


def kernel(x, w_q, w_kv_down, w_k_up, w_v_up, w_o):
    from concourse.bass_utils import run_bass_kernel_spmd

    nc = _get_module()
    in_maps = _host_inputs(x, w_q, w_kv_down, w_k_up, w_v_up, w_o)
    res = run_bass_kernel_spmd(nc, in_maps, list(range(N_CORES)))
    acc = np.zeros((T, C), np.float32)
    for c in range(N_CORES):
        acc += res.results[c]["out"].astype(np.float32)
    return acc.reshape(1, T, C)
